# revision 6
# baseline (speedup 1.0000x reference)
"""V3: single-descriptor combined-candidate-table gather.

Key = 256x256 grid cell (kx=floor(px*256), ky=floor(py*256)). Candidate bases
are affine in the key (L1: kx-1, L2: 2kx-1, L3: 4kx-1; L0: floor(127kx/256)
piecewise-affine), so one 256B entry holds every row a point in the cell can
need: 2x2 + 2x2 + 3x3 + 5x5 = 42 rows x 3 bf16 = 252B. One descriptor/point.
Entries indexed as int16 = key-32768 against a +32768-entry-biased base
(negative idx sign-extension verified on HW). The last 128 idx slots of each
gather are pad zeros so the ucode's trailing-negative trim never fires on
real points. Extraction: in-place sequential predicated copies per axis.
"""
import sys
sys.path.insert(0, '/opt/trn_rl_repo')
import numpy as np

from concourse import bass, bacc, mybir, library_config
from concourse.bass_utils import run_bass_kernel_spmd

N = 4194304
NCORES = 8
NC = N // NCORES
C = NC // 128            # 4096 point columns
CM = 512                 # columns per macro
NMACRO = C // CM         # 8
JCOL = 32                # real point columns per gather instr
J = 4224                 # idxs per instr = 33*128 (last 128 pad)
WPI = J // 16            # 264
IPM = CM // JCOL         # 16 instrs per macro
NINSTR = C // IPM // JCOL * IPM * NMACRO // NMACRO  # 128
NINSTR = C // JCOL       # 128
NGD = 8
LODS = [128, 256, 512, 1024]
O0, O1, O2, O3 = 0, 12, 24, 51
NENT = 65536
SR3, SR2, SR1, SR0 = 3087, 1542, 774, 396

_cached = {}


def _by0(k):
    return (127 * k) // 256


def _l0_runs(ky0, dy):
    """Arithmetic runs of by0(ky0+p)+dy over p: list of (p0, p1, row0)."""
    runs = []
    p0 = 0
    while p0 < 128:
        row = min(_by0(ky0 + p0) + dy, 127)
        p1 = p0 + 1
        while p1 < 128 and min(_by0(ky0 + p1) + dy, 127) == row + (p1 - p0):
            p1 += 1
        runs.append((p0, p1, row))
        p0 = p1
    return runs


def _stg_count(b):
    n = 3 * 16 + 2 * 16 + 2 * 16 if b == 0 else 2 * 16 + 2 * 16 + 1 * 16
    n += 2 * 16 + 2 * 16 + 1 * 16      # dy-tail L3(2), L2(2), L1(1)
    if b == 0:
        n += 2 * 16                     # L2 p=0 fixups (2 singles)
    for dy in range(2):
        n += 16 * len(_l0_runs(128 * b, dy))
    return n


def _build():
    if "nc" in _cached:
        return _cached["nc"]
    nc = bacc.Bacc("TRN2", target_bir_lowering=False, num_swdge_queues=4,
                   detect_race_conditions=False)
    pts = nc.dram_tensor("pts", [NC, 2], mybir.dt.float32, kind="ExternalInput")
    cbs = [nc.dram_tensor(f"cb{i}", [r * r, 3], mybir.dt.float32, kind="ExternalInput")
           for i, r in enumerate(LODS)]
    out = nc.dram_tensor("out", [NC, 3], mybir.dt.float32, kind="ExternalOutput")
    Q = [nc.dram_tensor(f"q{i}", [r * r * 3], mybir.dt.bfloat16) for i, r in enumerate(LODS)]
    CT = nc.dram_tensor("ct", [NENT, 128], mybir.dt.bfloat16)

    s_pin = nc.alloc_semaphore("s_pin")
    s_pad = nc.alloc_semaphore("s_pad")
    s_pout = nc.alloc_semaphore("s_pout")
    s_stg = nc.alloc_semaphore("s_stg")
    s_asm = nc.alloc_semaphore("s_asm")
    s_strip = nc.alloc_semaphore("s_strip")
    s_pts = nc.alloc_semaphore("s_pts")
    s_idx = nc.alloc_semaphore("s_idx")
    s_rep = nc.alloc_semaphore("s_rep")
    s_ext = nc.alloc_semaphore("s_ext")
    s_out = nc.alloc_semaphore("s_out")
    s_mz = nc.alloc_semaphore("s_mz")
    gsem = [nc.alloc_semaphore(f"g{q}") for q in range(4)]

    pts_sb = nc.alloc_sbuf_tensor("pts_sb", [128, 2 * CM], mybir.dt.float32)
    fa = nc.alloc_sbuf_tensor("fa", [128, CM], mybir.dt.float32)
    fb = nc.alloc_sbuf_tensor("fb", [128, CM], mybir.dt.float32)
    fx = nc.alloc_sbuf_tensor("fx", [128, CM], mybir.dt.float32)
    fy = nc.alloc_sbuf_tensor("fy", [128, CM], mybir.dt.float32)
    fr = nc.alloc_sbuf_tensor("fr", [128, CM], mybir.dt.float32)
    kxf = nc.alloc_sbuf_tensor("kxf", [128, CM], mybir.dt.float32)
    kyf = nc.alloc_sbuf_tensor("kyf", [128, CM], mybir.dt.float32)
    msk = [nc.alloc_sbuf_tensor(f"msk{s}", [128, 16 * CM], mybir.dt.uint8) for s in range(2)]
    wbuf = nc.alloc_sbuf_tensor("wbuf", [128, IPM * WPI], mybir.dt.int16)
    wtmp = nc.alloc_sbuf_tensor("wtmp", [128, IPM * WPI], mybir.dt.int16)
    gd = [nc.alloc_sbuf_tensor(f"gd{b}", [128, (J // 128) * 128], mybir.dt.bfloat16)
          for b in range(NGD)]
    outm = [nc.alloc_sbuf_tensor(f"outm{b}", [128, 3 * CM], mybir.dt.float32) for b in range(2)]
    stg3 = nc.alloc_sbuf_tensor("stg3", [128, 5 * SR3], mybir.dt.bfloat16)
    stg2 = nc.alloc_sbuf_tensor("stg2", [128, 3 * SR2], mybir.dt.bfloat16)
    stg1 = nc.alloc_sbuf_tensor("stg1", [128, 2 * SR1], mybir.dt.bfloat16)
    stg0 = nc.alloc_sbuf_tensor("stg0", [128, 2 * SR0], mybir.dt.bfloat16)
    strip = nc.alloc_sbuf_tensor("strip", [128, 128 * 128], mybir.dt.bfloat16)
    warm = nc.alloc_sbuf_tensor("warm", [128, 16], mybir.dt.float32)

    cbuf_f = strip[:].bitcast(mybir.dt.float32)   # [128, 8192] f32 chunk in
    cbout = stg3[:]                               # bf16 chunk out (>=8192)
    casts = []
    for l, r in enumerate(LODS):
        X = r * r * 3 // 128
        done = 0
        while done < X:
            step = min(8192, X - done)
            casts.append((l, done, step))
            done += step

    with nc.Block() as block:
        # ================= sync =================
        @block.sync
        def _(s):
            npout = 0
            for i, (l, off, step) in enumerate(casts):
                src = cbs[l][:].rearrange("(p x) f -> p (x f)", p=128)
                s.dma_start(out=cbuf_f[:, :step],
                            in_=src[:, off:off + step]).then_inc(s_pin, 16)
                s.wait_ge(s_pad, i + 1)
                dstv = Q[l][:].rearrange("(p x) -> p x", p=128)
                s.dma_start(out=dstv[:, off:off + step],
                            in_=cbout[:, :step]).then_inc(s_pout, 16)
                npout += 16
                s.wait_ge(s_pout, npout)

            nstrip = 0
            for b in range(2):
                ky0 = 128 * b
                if b == 1:
                    s.wait_ge(s_asm, 2)
                q3, q2, q1, q0 = Q[3][:], Q[2][:], Q[1][:], Q[0][:]
                s3 = stg3[:].rearrange("p (dy x) -> p dy x", x=SR3)
                s2 = stg2[:].rearrange("p (dy x) -> p dy x", x=SR2)
                s1 = stg1[:].rearrange("p (dy x) -> p dy x", x=SR1)
                s0 = stg0[:].rearrange("p (dy x) -> p dy x", x=SR0)
                if b == 0:
                    iv = q3[9216:9216 + 127 * 12288].rearrange(
                        "(p dy x) -> p dy x", dy=4, x=3072)
                    s.dma_start(out=s3[1:128, 0:4, 3:3075], in_=iv).then_inc(s_stg, 16)
                    s.dma_start(out=s3[0:1, 0, 3:3075],
                                in_=q3[0:3072].rearrange("(o x) -> o x", o=1)).then_inc(s_stg, 16)
                    iv = q3[0:3 * 3072].rearrange("(o dy x) -> o dy x", o=1, x=3072)
                    s.dma_start(out=s3[0:1, 1:4, 3:3075], in_=iv).then_inc(s_stg, 16)
                    iv = q2[1536:1536 + 127 * 3072].rearrange(
                        "(p dy x) -> p dy x", dy=2, x=1536)
                    s.dma_start(out=s2[1:128, 0:2, 3:1539], in_=iv).then_inc(s_stg, 16)
                    s.dma_start(out=s2[0:1, 0, 3:1539],
                                in_=q2[0:1536].rearrange("(o x) -> o x", o=1)).then_inc(s_stg, 16)
                    s.dma_start(out=s2[0:1, 1, 3:1539],
                                in_=q2[0:1536].rearrange("(o x) -> o x", o=1)).then_inc(s_stg, 16)
                    iv = q1[0:127 * 768].rearrange("(p x) -> p x", x=768)
                    s.dma_start(out=s1[1:128, 0, 3:771], in_=iv).then_inc(s_stg, 16)
                    s.dma_start(out=s1[0:1, 0, 3:771],
                                in_=q1[0:768].rearrange("(o x) -> o x", o=1)).then_inc(s_stg, 16)
                else:
                    base3 = (4 * 128 - 1) * 3072
                    iv = q3[base3:base3 + 127 * 12288].rearrange(
                        "(p dy x) -> p dy x", dy=4, x=3072)
                    s.dma_start(out=s3[0:127, 0:4, 3:3075], in_=iv).then_inc(s_stg, 16)
                    tl = base3 + 127 * 12288
                    iv = q3[tl:tl + 4 * 3072].rearrange("(o dy x) -> o dy x", o=1, x=3072)
                    s.dma_start(out=s3[127:128, 0:4, 3:3075], in_=iv).then_inc(s_stg, 16)
                    base2 = (2 * 128 - 1) * 1536
                    iv = q2[base2:base2 + 127 * 3072].rearrange(
                        "(p dy x) -> p dy x", dy=2, x=1536)
                    s.dma_start(out=s2[0:127, 0:2, 3:1539], in_=iv).then_inc(s_stg, 16)
                    tl = base2 + 127 * 3072
                    iv = q2[tl:tl + 2 * 1536].rearrange("(o dy x) -> o dy x", o=1, x=1536)
                    s.dma_start(out=s2[127:128, 0:2, 3:1539], in_=iv).then_inc(s_stg, 16)
                    base1 = 127 * 768
                    iv = q1[base1:base1 + 128 * 768].rearrange("(p x) -> p x", x=768)
                    s.dma_start(out=s1[:, 0, 3:771], in_=iv).then_inc(s_stg, 16)
                # dy tails
                base = (4 * ky0 + 3) * 3072
                iv = q3[base:base + 127 * 12288].rearrange("(p x) -> p x", x=12288)[:, 0:3072]
                s.dma_start(out=s3[0:127, 4, 3:3075], in_=iv).then_inc(s_stg, 16)
                tl = base + 127 * 12288
                s.dma_start(out=s3[127:128, 4, 3:3075],
                            in_=q3[tl:tl + 3072].rearrange("(o x) -> o x", o=1)).then_inc(s_stg, 16)
                base = (2 * ky0 + 1) * 1536
                iv = q2[base:base + 127 * 3072].rearrange("(p x) -> p x", x=3072)[:, 0:1536]
                s.dma_start(out=s2[0:127, 2, 3:1539], in_=iv).then_inc(s_stg, 16)
                tl = base + 127 * 3072
                s.dma_start(out=s2[127:128, 2, 3:1539],
                            in_=q2[tl:tl + 1536].rearrange("(o x) -> o x", o=1)).then_inc(s_stg, 16)
                base = ky0 * 768
                iv = q1[base:base + 128 * 768].rearrange("(p x) -> p x", x=768)
                s.dma_start(out=s1[:, 1, 3:771], in_=iv).then_inc(s_stg, 16)
                # L0 runs
                for dy in range(2):
                    for (p0, p1, row) in _l0_runs(ky0, dy):
                        npart = p1 - p0
                        iv = q0[row * 384:(row + npart) * 384].rearrange("(p x) -> p x", x=384)
                        s.dma_start(out=s0[p0:p1, dy, 3:387], in_=iv).then_inc(s_stg, 16)
                # strip halves out
                ctr = CT[:].rearrange("(ky two k) e -> ky two (k e)", two=2, k=128)
                for h in range(2):
                    s.wait_ge(s_asm, 2 * b + h + 1)
                    s.dma_start(out=ctr[ky0:ky0 + 128, h, :],
                                in_=strip[:, 0:16384]).then_inc(s_strip, 16)
                    nstrip += 16
            s.wait_ge(s_strip, nstrip)

            pv = pts[:].rearrange("(p c) t -> p (c t)", p=128)
            ovx = out[:].rearrange("(p c) t -> p (c t)", p=128)
            for m in range(NMACRO):
                s.dma_start(out=pts_sb[:],
                            in_=pv[:, 2 * m * CM:2 * (m + 1) * CM]).then_inc(s_pts, 16)
                s.wait_ge(s_idx, m + 1)
                for g in range(4):
                    s.dma_start(out=wbuf[32 * g:32 * (g + 1), :],
                                in_=wtmp[0:32, :]).then_inc(s_rep, 16)
                s.wait_ge(s_ext, IPM * (m + 1))
                s.dma_start(out=ovx[:, 3 * m * CM:3 * (m + 1) * CM],
                            in_=outm[m % 2][:]).then_inc(s_out, 16)
            s.wait_ge(s_out, 16 * NMACRO)

        # ================= vector =================
        @block.vector
        def _(v):
            v.memset(wbuf[:], 0)
            v.memset(wtmp[:], 0)
            v.drain()
            for i, (l, off, step) in enumerate(casts):
                v.wait_ge(s_pin, 16 * (i + 1))
                v.tensor_copy(out=cbout[:, :step], in_=cbuf_f[:, :step])
                v.drain().then_inc(s_pad, 1)
                if i + 1 < len(casts):
                    v.wait_ge(s_pout, 16 * (i + 1))

            sv = strip[:].rearrange("p (k e) -> p k e", e=128)
            sv2 = strip[:].rearrange("p (sk two e) -> p sk two e", two=2, e=128)
            s3 = stg3[:].rearrange("p (dy x) -> p dy x", x=SR3)
            s2 = stg2[:].rearrange("p (dy x) -> p dy x", x=SR2)
            s1 = stg1[:].rearrange("p (dy x) -> p dy x", x=SR1)
            s0 = stg0[:].rearrange("p (dy x) -> p dy x", x=SR0)
            run_stg = 0
            for b in range(2):
                run_stg += _stg_count(b)
                v.wait_ge(s_stg, run_stg)
                for dy in range(5):
                    v.tensor_copy(out=s3[:, dy, 0:3], in_=s3[:, dy, 3:6])
                for dy in range(3):
                    v.tensor_copy(out=s2[:, dy, 0:3], in_=s2[:, dy, 3:6])
                for dy in range(2):
                    v.tensor_copy(out=s1[:, dy, 0:3], in_=s1[:, dy, 3:6])
                v.drain()
                for h in range(2):
                    if 2 * b + h >= 1:
                        v.wait_ge(s_strip, 16 * (2 * b + h))
                    for dy in range(5):
                        for dx in range(5):
                            so = O3 + (dy * 5 + dx) * 3
                            ib = (4 * 128 * h + dx) * 3
                            iv = s3[:, dy, ib:ib + 12 * 128].rearrange(
                                "p (k e) -> p k e", e=12)[:, :, 0:3]
                            v.tensor_copy(out=sv[:, :, so:so + 3], in_=iv)
                    for dy in range(3):
                        for dx in range(3):
                            so = O2 + (dy * 3 + dx) * 3
                            ib = (2 * 128 * h + dx) * 3
                            iv = s2[:, dy, ib:ib + 6 * 128].rearrange(
                                "p (k e) -> p k e", e=6)[:, :, 0:3]
                            v.tensor_copy(out=sv[:, :, so:so + 3], in_=iv)
                    for dy in range(2):
                        for dx in range(2):
                            so = O1 + (dy * 2 + dx) * 3
                            ib = (128 * h + dx) * 3
                            iv = s1[:, dy, ib:ib + 3 * 128].rearrange(
                                "p (k e) -> p k e", e=3)[:, :, 0:3]
                            v.tensor_copy(out=sv[:, :, so:so + 3], in_=iv)
                    for dy in range(2):
                        for dx in range(2):
                            so = O0 + (dy * 2 + dx) * 3
                            if h == 0:
                                ib = dx * 3 + 3
                                v.tensor_copy(out=sv2[:, 0, 0, so:so + 3],
                                              in_=s0[:, dy, ib:ib + 3])
                                iv = s0[:, dy, ib:ib + 3 * 63].rearrange(
                                    "p (k e) -> p k e", e=3)
                                v.tensor_copy(out=sv2[:, 1:64, 0, so:so + 3], in_=iv)
                                iv = s0[:, dy, ib:ib + 3 * 64].rearrange(
                                    "p (k e) -> p k e", e=3)
                                v.tensor_copy(out=sv2[:, 0:64, 1, so:so + 3], in_=iv)
                            else:
                                ib = (63 + dx) * 3 + 3
                                iv = s0[:, dy, ib:ib + 3 * 64].rearrange(
                                    "p (k e) -> p k e", e=3)
                                v.tensor_copy(out=sv2[:, 0:64, 0, so:so + 3], in_=iv)
                                v.tensor_copy(out=sv2[:, 0:64, 1, so:so + 3], in_=iv)
                    v.drain().then_inc(s_asm, 1)

            # ---- main ----
            xv = pts_sb[:].rearrange("p (c t) -> p c t", t=2)
            i32 = fb[:].bitcast(mybir.dt.int32)
            for m in range(NMACRO):
                v.wait_ge(s_pts, 16 * (m + 1))
                ms = msk[m % 2][:].rearrange("p (s c) -> p s c", c=CM)
                for axis in range(2):
                    px = xv[:, :, axis]
                    kf = kxf if axis == 0 else kyf
                    v.tensor_scalar(out=fx[:], in0=px, scalar1=256.0, scalar2=-0.5,
                                    op0=mybir.AluOpType.mult, op1=mybir.AluOpType.add)
                    v.drain()
                    v.tensor_copy(out=i32, in_=fx[:])
                    v.drain()
                    v.tensor_copy(out=kf[:], in_=i32)
                    v.drain()
                    for l, r in enumerate(LODS):
                        v.tensor_scalar_mul(out=fx[:], in0=px, scalar1=float(r - 1))
                        v.drain()
                        v.tensor_copy(out=i32, in_=fx[:])
                        v.drain()
                        v.tensor_copy(out=fa[:], in_=i32)
                        v.drain()
                        v.tensor_sub(out=fr[:], in0=fa[:], in1=fx[:])
                        v.drain()
                        v.tensor_scalar(out=fr[:], in0=fr[:], scalar1=0.0, scalar2=None,
                                        op0=mybir.AluOpType.is_gt)
                        v.drain()
                        v.tensor_sub(out=fa[:], in0=fa[:], in1=fr[:])
                        v.drain()
                        if l == 0:
                            v.tensor_scalar_mul(out=fx[:], in0=kf[:], scalar1=127.0 / 256.0)
                            v.drain()
                            v.tensor_copy(out=i32, in_=fx[:])
                            v.drain()
                            v.tensor_copy(out=fy[:], in_=i32)
                            v.drain()
                            v.tensor_sub(out=fr[:], in0=fy[:], in1=fx[:])
                            v.drain()
                            v.tensor_scalar(out=fr[:], in0=fr[:], scalar1=0.0, scalar2=None,
                                            op0=mybir.AluOpType.is_gt)
                            v.drain()
                            v.tensor_sub(out=fy[:], in0=fy[:], in1=fr[:])
                            v.drain()
                            v.tensor_sub(out=fr[:], in0=fa[:], in1=fy[:])
                            v.drain()
                            v.tensor_scalar(out=ms[:, axis, :], in0=fr[:], scalar1=0.5,
                                            scalar2=None, op0=mybir.AluOpType.is_ge)
                            v.drain()
                        else:
                            c = [None, 1.0, 2.0, 4.0][l]
                            v.scalar_tensor_tensor(out=fr[:], in0=kf[:], scalar=-c,
                                                   in1=fa[:], op0=mybir.AluOpType.mult,
                                                   op1=mybir.AluOpType.add)
                            v.drain()
                            nmask = [0, 1, 2, 4][l]
                            mbase = {1: 2, 2: 4, 3: 8}[l] + axis * nmask
                            for k in range(nmask):
                                v.tensor_scalar(out=ms[:, mbase + k, :], in0=fr[:],
                                                scalar1=float(k) - 0.5, scalar2=None,
                                                op0=mybir.AluOpType.is_ge)
                                v.drain()
                v.scalar_tensor_tensor(out=fx[:], in0=kyf[:], scalar=256.0, in1=kxf[:],
                                       op0=mybir.AluOpType.mult, op1=mybir.AluOpType.add)
                v.drain()
                v.tensor_scalar(out=fx[:], in0=fx[:], scalar1=-32768.0, scalar2=None,
                                op0=mybir.AluOpType.add)
                v.drain()
                if m >= 1:
                    v.wait_ge(s_rep, 64 * m)
                wr = wbuf[:].rearrange("p (i w) -> p i w", w=WPI)[:, :, 0:256] \
                    .rearrange("p i (cc q) -> p i cc q", q=8)
                fxv = fx[:].rearrange("p (i cc) -> p i cc", cc=JCOL)
                for q in (0, 2, 4, 6):
                    v.tensor_copy(out=wr[0:16, :, :, q], in_=fxv[16 * q:16 * (q + 1), :, :])
                v.stream_shuffle(out=fy[:], in_=fx[:], mask=[(i + 16) % 32 for i in range(32)])
                v.drain()
                fyv = fy[:].rearrange("p (i cc) -> p i cc", cc=JCOL)
                for q in (1, 3, 5, 7):
                    v.tensor_copy(out=wr[0:16, :, :, q],
                                  in_=fyv[16 * (q - 1):16 * (q - 1) + 16, :, :])
                v.drain()
                v.stream_shuffle(out=wtmp[0:32, :], in_=wbuf[0:32, :],
                                 mask=[i % 16 for i in range(32)])
                v.drain().then_inc(s_idx, 1)

                if m >= 2:
                    v.wait_ge(s_out, 16 * (m - 1))
                for il in range(IPM):
                    gi = m * IPM + il
                    bidx = gi % NGD
                    v.wait_ge(gsem[gi % 4], 16 * (gi // 4 + 1))
                    gv = gd[bidx][:].rearrange("p (cc e) -> p cc e", e=128)[:, 0:JCOL, :]
                    c0 = il * JCOL

                    def mk(slot, dims):
                        mc = ms[:, slot, c0:c0 + JCOL]
                        for _ in range(len(dims) - 2):
                            mc = mc.unsqueeze(-1)
                        return mc.to_broadcast(dims)

                    l0 = gv[:, :, O0:O0 + 12].rearrange("p cc (dy x) -> p cc dy x", x=6)
                    v.copy_predicated(out=l0[:, :, :, 0:3], mask=mk(0, [128, JCOL, 2, 3]),
                                      data=l0[:, :, :, 3:6])
                    l1 = gv[:, :, O1:O1 + 12].rearrange("p cc (dy x) -> p cc dy x", x=6)
                    v.copy_predicated(out=l1[:, :, :, 0:3], mask=mk(2, [128, JCOL, 2, 3]),
                                      data=l1[:, :, :, 3:6])
                    l2 = gv[:, :, O2:O2 + 27].rearrange("p cc (dy x) -> p cc dy x", x=9)
                    for k in range(2):
                        v.copy_predicated(out=l2[:, :, :, 0:3],
                                          mask=mk(4 + k, [128, JCOL, 3, 3]),
                                          data=l2[:, :, :, 3 * (k + 1):3 * (k + 1) + 3])
                    l3 = gv[:, :, O3:O3 + 75].rearrange("p cc (dy x) -> p cc dy x", x=15)
                    for k in range(4):
                        v.copy_predicated(out=l3[:, :, :, 0:3],
                                          mask=mk(8 + k, [128, JCOL, 5, 3]),
                                          data=l3[:, :, :, 3 * (k + 1):3 * (k + 1) + 3])
                    v.drain()
                    v.copy_predicated(out=gv[:, :, O0:O0 + 3], mask=mk(1, [128, JCOL, 3]),
                                      data=gv[:, :, O0 + 6:O0 + 9])
                    v.copy_predicated(out=gv[:, :, O1:O1 + 3], mask=mk(3, [128, JCOL, 3]),
                                      data=gv[:, :, O1 + 6:O1 + 9])
                    for k in range(2):
                        v.copy_predicated(out=gv[:, :, O2:O2 + 3],
                                          mask=mk(6 + k, [128, JCOL, 3]),
                                          data=gv[:, :, O2 + 9 * (k + 1):O2 + 9 * (k + 1) + 3])
                    for k in range(4):
                        v.copy_predicated(out=gv[:, :, O3:O3 + 3],
                                          mask=mk(12 + k, [128, JCOL, 3]),
                                          data=gv[:, :, O3 + 15 * (k + 1):O3 + 15 * (k + 1) + 3])
                    v.drain()
                    fav = fa[:, 0:3 * JCOL].rearrange("p (cc f) -> p cc f", f=3)
                    fbv = fb[:, 0:3 * JCOL].rearrange("p (cc f) -> p cc f", f=3)
                    v.tensor_add(out=fav, in0=gv[:, :, O0:O0 + 3], in1=gv[:, :, O1:O1 + 3])
                    v.tensor_add(out=fbv, in0=gv[:, :, O2:O2 + 3], in1=gv[:, :, O3:O3 + 3])
                    v.drain()
                    ov = outm[m % 2][:, 3 * c0:3 * c0 + 3 * JCOL].rearrange(
                        "p (cc f) -> p cc f", f=3)
                    v.tensor_add(out=ov, in0=fav, in1=fbv)
                    v.drain().then_inc(s_ext, 1)

        # ================= gpsimd =================
        @block.gpsimd
        def _(gp):
            gp.load_library(library_config.mlp)
            gp.memzero(warm[:]).then_inc(s_mz, 1)
            gp.wait_ge(s_mz, 1)
            gp.wait_ge(s_strip, 64)
            ctv = CT[32768:, :]
            for gi in range(NINSTR):
                m = gi // IPM
                il = gi % IPM
                if il == 0:
                    gp.wait_ge(s_idx, m + 1)
                    gp.wait_ge(s_rep, 64 * (m + 1))
                if gi >= NGD:
                    gp.wait_ge(s_ext, gi - NGD + 1)
                q = gi % 4
                if gi >= 4:
                    gp.wait_ge(gsem[q], 16 * (gi // 4))
                gp.dma_gather(
                    out_ap=gd[gi % NGD][:].rearrange("p (cc e) -> p cc e", e=128),
                    in_ap=ctv,
                    idxs_ap=wbuf[:, il * WPI:(il + 1) * WPI],
                    num_idxs=J, num_idxs_reg=J, elem_size=128,
                    queue_num=q).then_inc(gsem[q], 16)
    nc.compile()
    _cached["nc"] = nc
    return nc


def _make_in_maps(inputs):
    pts = np.ascontiguousarray(inputs["pts"], dtype=np.float32)
    cbsv = [np.ascontiguousarray(inputs[f"cb{i}"], dtype=np.float32) for i in range(4)]
    in_maps = []
    for c in range(NCORES):
        in_maps.append({
            "pts": pts[c * NC:(c + 1) * NC],
            "cb0": cbsv[0], "cb1": cbsv[1], "cb2": cbsv[2], "cb3": cbsv[3],
        })
    return in_maps


def kernel(pts, cb0, cb1, cb2, cb3):
    nc = _build()
    in_maps = _make_in_maps(dict(pts=pts, cb0=cb0, cb1=cb1, cb2=cb2, cb3=cb3))
    res = run_bass_kernel_spmd(nc, in_maps, list(range(NCORES)))
    return np.concatenate([res.results[c]["out"] for c in range(NCORES)], axis=0)
